# Initial kernel scaffold
#
"""Trainium2 Bass kernel for nn_AttentionBlock (B=2,T=2048,C=1024,H=16,D=64,F=4096).

Sharding: 8 cores = 2 batches x 4 query-chunks of 512 tokens. Each core
computes K/V for the full sequence of its batch (replicated within the
4-core batch group), and attention + MLP for its own 512 query tokens.
All matmuls run in bf16 with f32 PSUM accumulation; the residual stream,
softmax denominators and rms statistics stay in f32.

Self-contained: hardcodes shapes; host-side prep slices/casts/transposes
inputs per core, device output is [C, TQ] per core, host reassembles.
"""
import os
from contextlib import ExitStack

import numpy as np
import ml_dtypes

BF16 = ml_dtypes.bfloat16

B, T, C, H, D, F = 2, 2048, 1024, 16, 64, 4096
NCORES, G = 8, 4
TQ = T // G                 # 512 query tokens per core
HD = H * D                  # 1024
EPS = 1e-8
CC = C // 128               # 8 channel chunks
TC = T // 128               # 16 token chunks
FC = F // 128               # 32 hidden chunks

# stream_shuffle permutes lanes within each 32-partition quadrant
# (out[s*32+i] = in[s*32+mask[i]]). We host-permute the head dim so rope
# partners (d, d+32) sit on adjacent partitions; the swap is then [1,0,3,2,..].
ROPE_PERM = np.arange(64).reshape(2, 32).T.reshape(-1)   # [0,32,1,33,...]
SWAP_MASK = [i ^ 1 for i in range(32)]

LAST_RESULTS = None  # BassKernelResults of the last run (for test.py)
USE_ALLGATHER = True  # shard K/V across the 4-core batch group + AllGather


def host_prep(inputs):
    x = np.asarray(inputs["x"], np.float32)
    alibi = np.asarray(inputs["alibi"], np.float32)
    rot = np.asarray(inputs["rotational"], np.float32)
    g_att = np.asarray(inputs["g_att"], np.float32)
    g_mlp = np.asarray(inputs["g_mlp"], np.float32)
    w_qkv = np.asarray(inputs["w_qkv"], np.float32)
    w_att_out = np.asarray(inputs["w_att_out"], np.float32)
    w_mlp_in = np.asarray(inputs["w_mlp_in"], np.float32)
    b_mlp_in = np.asarray(inputs["b_mlp_in"], np.float32)
    w_mlp_out = np.asarray(inputs["w_mlp_out"], np.float32)
    b_mlp_out = np.asarray(inputs["b_mlp_out"], np.float32)

    wg = w_qkv * g_att[None, :]                   # fold g_att into qkv
    wg[:HD] *= 1.0 / np.sqrt(np.float32(D))       # fold attn scale into w_q

    # permute each head's 64 q/k output dims so rope pairs are adjacent
    wqk_p = wg[:2 * HD].reshape(2, H, 64, C)[:, :, ROPE_PERM, :].reshape(
        2 * HD, C)
    wqkvT = np.ascontiguousarray(
        np.concatenate([wqk_p, wg[2 * HD:]], 0).T).astype(BF16)    # [C, 3HD]
    wqkT_t = np.ascontiguousarray(
        wqkvT[:, :2 * HD].reshape(C, 16, 128).transpose(1, 0, 2))  # [16,C,128]
    wvT_r = np.ascontiguousarray(
        wqkvT[:, 2 * HD:].reshape(CC, 128, HD))                    # [8,128,HD]
    woT = np.ascontiguousarray(w_att_out.T).astype(BF16)           # [HD, C]
    woT_t = np.ascontiguousarray(
        woT.reshape(8, 128, CC, 128).transpose(2, 0, 1, 3))        # [cc,hd,128,128]
    w_inT = np.ascontiguousarray((w_mlp_in * g_mlp[None, :]).T).astype(BF16)
    w_inT_t = np.ascontiguousarray(
        w_inT.reshape(C, FC, 128).transpose(1, 0, 2))              # [32,C,128]
    w_outT = np.ascontiguousarray(w_mlp_out.T).astype(BF16)        # [F, C]
    w_outT_t = np.ascontiguousarray(
        w_outT.reshape(FC, 128, CC, 128).transpose(0, 2, 1, 3))    # [f,cc,128,128]

    b_in_t = np.ascontiguousarray(b_mlp_in.reshape(FC, 128).T)     # [128, 32]
    b_out_t = np.ascontiguousarray(b_mlp_out.reshape(CC, 128).T)   # [128, 8]

    cosT = np.cos(rot).T.astype(np.float32)                        # [D, T]
    sinT = np.sin(rot).T.astype(np.float32)
    sgn = np.where(np.arange(D) < D // 2, -1.0, 1.0).astype(np.float32)
    ssinT = sinT * sgn[:, None]
    cosT = cosT[ROPE_PERM]                             # match head-dim perm
    # pre-swap the sign-sin rows: device computes swap(x * ss) == swap(x) * s
    ssinT = ssinT[ROPE_PERM][np.arange(64) ^ 1]
    cs2T = np.ascontiguousarray(np.tile(cosT, (2, 1))).astype(BF16)  # [128, T]
    ss2T = np.ascontiguousarray(np.tile(ssinT, (2, 1))).astype(BF16)

    xT = {b: np.ascontiguousarray(x[b].T) for b in range(B)}       # [C,T] f32
    return dict(alibi=alibi, wqkT_t=wqkT_t, wvT_r=wvT_r, woT_t=woT_t,
                w_inT_t=w_inT_t, w_outT_t=w_outT_t, b_in_t=b_in_t,
                b_out_t=b_out_t, cs2T=cs2T, ss2T=ss2T, xT=xT)


def core_inputs(hp, core):
    b, j = core // G, core % G
    q0 = j * TQ
    al = hp["alibi"][:, q0:q0 + TQ, :]                # [H, TQ, T]
    return dict(
        xT_own=np.ascontiguousarray(hp["xT"][b][:, q0:q0 + TQ]),  # f32 [C,TQ]
        xbf=hp["xT"][b].astype(BF16),                             # [C,T]
        xbf_own=np.ascontiguousarray(hp["xT"][b][:, q0:q0 + TQ]).astype(BF16),
        alibi_t=np.ascontiguousarray(al.transpose(0, 2, 1)).astype(BF16),
        cs2T=hp["cs2T"], ss2T=hp["ss2T"],
        cs2T_own=np.ascontiguousarray(hp["cs2T"][:, q0:q0 + TQ]),
        ss2T_own=np.ascontiguousarray(hp["ss2T"][:, q0:q0 + TQ]),
        wqkT_t=hp["wqkT_t"], wvT_r=hp["wvT_r"], woT_t=hp["woT_t"],
        w_inT_t=hp["w_inT_t"], w_outT_t=hp["w_outT_t"],
        b_in_t=hp["b_in_t"], b_out_t=hp["b_out_t"],
    )


def build(nc, tc, io, ctx, phases="all"):
    import concourse.bass as bass
    import concourse.mybir as mybir
    from concourse.bass import ts
    from concourse.masks import make_identity

    dt = mybir.dt
    AF = mybir.ActivationFunctionType
    OP = mybir.AluOpType
    f32, bf16 = dt.float32, dt.bfloat16

    def pool(name, bufs, space="SBUF"):
        return ctx.enter_context(tc.tile_pool(name=name, bufs=bufs, space=space))

    consts = pool("consts", 1)
    ident = consts.tile([128, 128], bf16, tag="ident", name="ident")
    make_identity(nc, ident[:, :])
    ones_col = consts.tile([128, 1], bf16, tag="ones", name="ones")
    nc.vector.memset(ones_col[:, :], 1.0)
    ones_f32 = consts.tile([128, 1], f32, tag="ones32", name="ones32")
    nc.vector.memset(ones_f32[:, :], 1.0)
    b_in_sb = consts.tile([128, FC], f32, tag="b_in", name="b_in")
    nc.sync.dma_start(b_in_sb[:, :], io["b_in_t"][:, :])
    b_out_sb = consts.tile([128, CC], f32, tag="b_out", name="b_out")
    nc.sync.dma_start(b_out_sb[:, :], io["b_out_t"][:, :])

    y1_pool = pool("y1", CC)
    y2_pool = pool("y2", CC)

    qkv_scope = ExitStack()

    def qpool(name, bufs, space="SBUF"):
        return qkv_scope.enter_context(
            tc.tile_pool(name=name, bufs=bufs, space=space))

    QT_pool = qpool("QT", CC)
    KT_pool = qpool("KT", CC)
    V_pool = qpool("V", TC)
    QT, KT, V = [], [], []

    with ExitStack() as p1:
        def ppool(name, bufs, space="SBUF"):
            return p1.enter_context(
                tc.tile_pool(name=name, bufs=bufs, space=space))

        xbf = []
        if not USE_ALLGATHER:
            xbf_pool = ppool("xbf", CC)
            for c in range(CC):
                t = xbf_pool.tile([128, T], bf16, tag="xbf", name="xbf")
                nc.sync.dma_start(t[:, :], io["xbf"][ts(c, 128), :])
                xbf.append(t)
        xon = ppool("xon", 1).tile([128 * 0 + 128, 0 * 128 + CC, TQ], bf16,
                                   tag="xon", name="xon")
        # xbf_own as [128, CC, TQ] single tile
        nc.sync.dma_start(
            xon[:, :, :],
            io["xbf_own"].rearrange("(cc p) q -> p cc q", p=128))

        csr_pool = ppool("csr", 1)
        cso_r = csr_pool.tile([128, TQ], bf16, tag="csor", name="csor")
        sso_r = csr_pool.tile([128, TQ], bf16, tag="ssor", name="ssor")
        if not USE_ALLGATHER:
            cs_r = csr_pool.tile([128, T], bf16, tag="csr", name="csr")
            ss_r = csr_pool.tile([128, T], bf16, tag="ssr", name="ssr")
            r_col = csr_pool.tile([128, TC], f32, tag="rcol", name="rcol")
        else:
            ro_col = csr_pool.tile([128, TQ // 128], f32, tag="rocol",
                                   name="rocol")

        stats_scope = ExitStack()
        rms1_pool = stats_scope.enter_context(tc.tile_pool(name="rms1", bufs=1))
        sq_pool = stats_scope.enter_context(tc.tile_pool(name="sq", bufs=2))
        rms1_ps = stats_scope.enter_context(
            tc.tile_pool(name="rms1ps", bufs=1, space="PSUM"))

        if not USE_ALLGATHER:
            # full-T rmsnorm stats from bf16 x
            ssq = [rms1_ps.tile([1, 512], f32, tag=f"ssq{i}", name=f"ssq{i}")
                   for i in range(4)]
            for c in range(CC):
                sq = sq_pool.tile([128, T], bf16, tag="sq", name="sq")
                nc.scalar.activation(sq[:, :], xbf[c][:, :], AF.Square)
                for n in range(4):
                    nc.tensor.matmul(ssq[n][:, :], ones_col[:, :],
                                     sq[:, ts(n, 512)],
                                     start=(c == 0), stop=(c == CC - 1))
            r_sb = rms1_pool.tile([1, T], f32, tag="r1", name="r1")
            r128 = rms1_pool.tile([128, T], f32, tag="r1b", name="r1b")
            for n in range(4):
                nc.scalar.activation(r128[0:1, ts(n, 512)], ssq[n][:, :],
                                     AF.Sqrt, bias=0.0, scale=1.0 / C)
            nc.vector.reciprocal(r_sb[:, :], r128[0:1, :])
            nc.gpsimd.partition_broadcast(r128[:, :], r_sb[:, :])

        # own-token rmsnorm stats
        ssqo = rms1_ps.tile([1, 512], f32, tag="ssqo", name="ssqo")
        for c in range(CC):
            sqo = sq_pool.tile([128, TQ], bf16, tag="sqo", name="sqo")
            nc.scalar.activation(sqo[:, :], xon[:, c, :], AF.Square)
            nc.tensor.matmul(ssqo[:, :], ones_col[:, :], sqo[:, :],
                             start=(c == 0), stop=(c == CC - 1))
        ro_sb = rms1_pool.tile([1, TQ], f32, tag="ro", name="ro")
        sdo_sb = rms1_pool.tile([1, TQ], f32, tag="sdo", name="sdo")
        nc.scalar.activation(sdo_sb[:, :], ssqo[:, :], AF.Sqrt,
                             bias=0.0, scale=1.0 / C)
        nc.vector.reciprocal(ro_sb[:, :], sdo_sb[:, :])
        ro128 = rms1_pool.tile([128, TQ], f32, tag="rob", name="rob")
        nc.gpsimd.partition_broadcast(ro128[:, :], ro_sb[:, :])

        # rope tables with r folded in
        with tc.tile_pool(name="cstmp", bufs=1) as cst:
            pairs = [(cso_r, "cs2T_own", ro128), (sso_r, "ss2T_own", ro128)]
            if not USE_ALLGATHER:
                pairs += [(cs_r, "cs2T", r128), (ss_r, "ss2T", r128)]
            for dst, src_name, rb in pairs:
                tmp = cst.tile(list(dst.shape), bf16, tag="cstmp", name="cstmp")
                nc.sync.dma_start(tmp[:, :], io[src_name][:, :])
                nc.vector.tensor_tensor(dst[:, :], tmp[:, :], rb[:, :], OP.mult)

        # transpose r -> per-token-chunk columns for the V scaling
        rt_ps = stats_scope.enter_context(
            tc.tile_pool(name="rtps", bufs=2, space="PSUM"))
        if USE_ALLGATHER:
            for t in range(TQ // 128):
                ps = rt_ps.tile([128, 1], f32, tag="rtps", name="rtps")
                nc.tensor.matmul(ps[:, :], ro_sb[0:1, ts(t, 128)],
                                 ones_f32[0:1, 0:1], start=True, stop=True)
                nc.vector.tensor_copy(ro_col[:, t:t + 1], ps[:, :])
        else:
            for t in range(TC):
                ps = rt_ps.tile([128, 1], f32, tag="rtps", name="rtps")
                nc.tensor.matmul(ps[:, :], r_sb[0:1, ts(t, 128)],
                                 ones_f32[0:1, 0:1], start=True, stop=True)
                nc.vector.tensor_copy(r_col[:, t:t + 1], ps[:, :])
        stats_scope.close()

        # ---------------- QKV projections + rope -----------------------
        wslab_pool = ppool("wslab", 3)
        qk_ps = ppool("qkps", 3, "PSUM")
        rope_pool = ppool("ropet", 6)

        def rope(dst_ap, ps, cs_ap, ss_ap):
            qc = rope_pool.tile([128, 512], bf16, tag="ropeA", name="ropeA")
            nc.vector.tensor_tensor(qc[:, :], ps[:, :], cs_ap, OP.mult)
            tmp = rope_pool.tile([128, 512], bf16, tag="ropeB", name="ropeB")
            nc.vector.tensor_tensor(tmp[:, :], ps[:, :], ss_ap, OP.mult)
            qs = rope_pool.tile([128, 512], bf16, tag="ropeC", name="ropeC")
            nc.vector.stream_shuffle(qs[:, :], tmp[:, :], SWAP_MASK)
            nc.vector.tensor_tensor(dst_ap, qc[:, :], qs[:, :], OP.add)

        if USE_ALLGATHER:
            NV = TQ // 128           # 4 own V chunks
            KO_ELEMS = C * TQ        # 8 chunks of [128, 512]
            VO_ELEMS = NV * 128 * 1040
            dram_pool = qpool("kvdram", 1, "DRAM")
            kv_own = dram_pool.tile([KO_ELEMS + VO_ELEMS], bf16,
                                    tag="kvown", name="kvown")
            kv_gath = dram_pool.tile([G * (KO_ELEMS + VO_ELEMS)], bf16,
                                     tag="kvgath", name="kvgath")
            for e in range(CC):      # K (own tokens), roped, to DRAM
                slab = wslab_pool.tile([128, CC, 128], bf16, tag="wslab",
                                       name="wslab")
                nc.sync.dma_start(
                    slab[:, :, :],
                    io["wqkT_t"][CC + e].rearrange("(cc p) m -> p cc m",
                                                   p=128))
                ps = qk_ps.tile([128, 512], f32, tag="qkps", name="qkps")
                for c in range(CC):
                    nc.tensor.matmul(ps[:, :], slab[:, c, :], xon[:, c, :],
                                     start=(c == 0), stop=(c == CC - 1))
                ko = rope_pool.tile([128, TQ], bf16, tag="ko", name="ko")
                rope(ko[:, :], ps, cso_r[:, :], sso_r[:, :])
                nc.sync.dma_start(
                    kv_own[e * 128 * TQ:(e + 1) * 128 * TQ].rearrange(
                        "(p q) -> p q", q=TQ), ko[:, :])
            # V (own tokens), padded per-head layout
            wv_pool = ppool("wvr", CC)
            wv = []
            for c in range(CC):
                t = wv_pool.tile([128, HD], bf16, tag="wvr", name="wvr")
                nc.sync.dma_start(t[:, :], io["wvT_r"][c][:, :])
                wv.append(t)
            for tcn in range(NV):
                vt = rope_pool.tile([128, 16 * 65], bf16, tag="vo", name="vo")
                vt3 = vt.rearrange("p (h x) -> p h x", x=65)
                nc.vector.memset(vt3[:, :, 64:65], 1.0)
                for n in range(2):
                    ps = qk_ps.tile([128, 512], f32, tag="qkps", name="qkps")
                    for c in range(CC):
                        nc.tensor.matmul(ps[:, :], xon[:, c, ts(tcn, 128)],
                                         wv[c][:, ts(n, 512)],
                                         start=(c == 0), stop=(c == CC - 1))
                    nc.vector.tensor_scalar_mul(
                        vt3[:, ts(n, 8), 0:64],
                        ps.rearrange("p (h d) -> p h d", d=64),
                        ro_col[:, tcn:tcn + 1])
                nc.sync.dma_start(
                    kv_own[KO_ELEMS + tcn * 128 * 1040:
                           KO_ELEMS + (tcn + 1) * 128 * 1040].rearrange(
                        "(p q) -> p q", q=1040), vt[:, :])
            for e in range(CC):      # Q (own tokens) — overlaps the gather
                slab = wslab_pool.tile([128, CC, 128], bf16, tag="wslab",
                                       name="wslab")
                nc.sync.dma_start(
                    slab[:, :, :],
                    io["wqkT_t"][e].rearrange("(cc p) m -> p cc m", p=128))
                ps = qk_ps.tile([128, 512], f32, tag="qkps", name="qkps")
                for c in range(CC):
                    nc.tensor.matmul(ps[:, :], slab[:, c, :], xon[:, c, :],
                                     start=(c == 0), stop=(c == CC - 1))
                qt = QT_pool.tile([128, TQ], bf16, tag="QT", name="QT")
                rope(qt[:, :], ps, cso_r[:, :], sso_r[:, :])
                QT.append(qt)
            # gather within the 4-core batch group
            nc.gpsimd.collective_compute(
                "AllGather", mybir.AluOpType.bypass,
                replica_groups=[[0, 1, 2, 3], [4, 5, 6, 7]],
                ins=[kv_own[:].opt()], outs=[kv_gath[:].opt()])
            _kv_handles = (kv_gath, KO_ELEMS, VO_ELEMS, NV)
        else:
            for e in range(CC):          # Q (own tokens)
                slab = wslab_pool.tile([128, CC, 128], bf16, tag="wslab",
                                       name="wslab")
                nc.sync.dma_start(
                    slab[:, :, :],
                    io["wqkT_t"][e].rearrange("(cc p) m -> p cc m", p=128))
                ps = qk_ps.tile([128, 512], f32, tag="qkps", name="qkps")
                for c in range(CC):
                    nc.tensor.matmul(ps[:, :], slab[:, c, :], xon[:, c, :],
                                     start=(c == 0), stop=(c == CC - 1))
                qt = QT_pool.tile([128, TQ], bf16, tag="QT", name="QT")
                rope(qt[:, :], ps, cso_r[:, :], sso_r[:, :])
                QT.append(qt)
            for e in range(CC):          # K (full T)
                slab = wslab_pool.tile([128, CC, 128], bf16, tag="wslab",
                                       name="wslab")
                nc.sync.dma_start(
                    slab[:, :, :],
                    io["wqkT_t"][CC + e].rearrange("(cc p) m -> p cc m",
                                                   p=128))
                kt = KT_pool.tile([128, T], bf16, tag="KT", name="KT")
                for n in range(4):
                    ps = qk_ps.tile([128, 512], f32, tag="qkps", name="qkps")
                    for c in range(CC):
                        nc.tensor.matmul(ps[:, :], slab[:, c, :],
                                         xbf[c][:, ts(n, 512)],
                                         start=(c == 0), stop=(c == CC - 1))
                    rope(kt[:, ts(n, 512)], ps, cs_r[:, ts(n, 512)],
                         ss_r[:, ts(n, 512)])
                KT.append(kt)

            # V (full T), padded layout [128, 16*65]
            wv_pool = ppool("wvr", CC)
            wv = []
            for c in range(CC):
                t = wv_pool.tile([128, HD], bf16, tag="wvr", name="wvr")
                nc.sync.dma_start(t[:, :], io["wvT_r"][c][:, :])
                wv.append(t)
            for tcn in range(TC):
                vt = V_pool.tile([128, 16 * 65], bf16, tag="V", name="V")
                vt3 = vt.rearrange("p (h x) -> p h x", x=65)
                nc.vector.memset(vt3[:, :, 64:65], 1.0)
                for n in range(2):
                    ps = qk_ps.tile([128, 512], f32, tag="qkps", name="qkps")
                    for c in range(CC):
                        nc.tensor.matmul(ps[:, :], xbf[c][:, ts(tcn, 128)],
                                         wv[c][:, ts(n, 512)],
                                         start=(c == 0), stop=(c == CC - 1))
                    nc.vector.tensor_scalar_mul(
                        vt3[:, ts(n, 8), 0:64],
                        ps.rearrange("p (h d) -> p h d", d=64),
                        r_col[:, tcn:tcn + 1])
                V.append(vt)

    al_scope = ExitStack()
    al_pool = al_scope.enter_context(tc.tile_pool(name="alibi", bufs=3))

    def load_alibi(h):
        al = al_pool.tile([128, TC * 512], bf16, tag="alibi", name="alibi")
        src3 = io["alibi_t"][h].rearrange("(tc p) q -> p tc q", p=128)
        al3 = al.rearrange("p (tc q) -> p tc q", q=512)
        for dd in range(4):
            nc.sync.dma_start(al3[:, ts(dd, 4), :], src3[:, ts(dd, 4), :])
        return al

    al_cache = {h: load_alibi(h) for h in (0, 1, 2)}

    if USE_ALLGATHER:
        kv_gath, KO_ELEMS, VO_ELEMS, NV = _kv_handles
        RK = KO_ELEMS + VO_ELEMS
        for e in range(CC):      # assemble KT [128, T]
            kt = KT_pool.tile([128, T], bf16, tag="KT", name="KT")
            for r in range(G):
                nc.sync.dma_start(
                    kt[:, r * TQ:(r + 1) * TQ],
                    kv_gath[r * RK + e * 128 * TQ:
                            r * RK + (e + 1) * 128 * TQ].rearrange(
                        "(p q) -> p q", q=TQ))
            KT.append(kt)
        for tkc in range(TC):    # assemble V tiles [128, 1040]
            r, sub = tkc // NV, tkc % NV
            vt = V_pool.tile([128, 16 * 65], bf16, tag="V", name="V")
            nc.sync.dma_start(
                vt[:, :],
                kv_gath[r * RK + KO_ELEMS + sub * 128 * 1040:
                        r * RK + KO_ELEMS + (sub + 1) * 128 * 1040
                        ].rearrange("(p q) -> p q", q=1040))
            V.append(vt)

    if phases == "qkv":
        qkv_scope.close()
        return

    # ---------------- attention ---------------------------------------
    yatt_scope = ExitStack()
    yatt_pool = yatt_scope.enter_context(tc.tile_pool(name="yatt", bufs=CC))
    yatt = [yatt_pool.tile([128, TQ], bf16, tag="yatt", name="yatt")
            for _ in range(CC)]

    with tc.tile_pool(name="scps", bufs=2, space="PSUM") as sc_ps, \
         tc.tile_pool(name="avps", bufs=4, space="PSUM") as av_ps, \
         tc.tile_pool(name="expt", bufs=6) as e_pool, \
         tc.tile_pool(name="attsm", bufs=4) as sm_pool, \
         tc.tile_pool(name="scadd", bufs=3) as sca_pool:

        for hp in range(8):
            h0, h1 = 2 * hp, 2 * hp + 1
            als = []
            for h in (h0, h1):
                al = al_cache.pop(h, None)
                if al is None:
                    al = load_alibi(h)
                als.append(al)
                nxt = h + 2
                if nxt < H and nxt not in al_cache and hp < 7:
                    pass  # pool bufs limit prefetch depth naturally
            av0 = av_ps.tile([65, 512], f32, tag="av", name="av0")
            av1 = av_ps.tile([65, 512], f32, tag="av", name="av1")
            for g in range(8):
                ps0 = sc_ps.tile([128, 1024], f32, tag="scps", name="scps0")
                ps1 = sc_ps.tile([128, 1024], f32, tag="scps", name="scps1")
                for s in range(2):
                    tkc = 2 * g + s
                    nc.tensor.matmul(ps0[:, ts(s, 512)],
                                     KT[hp][0:64, ts(tkc, 128)],
                                     QT[hp][0:64, :],
                                     start=True, stop=True,
                                     tile_position=(0, 0))
                    nc.tensor.matmul(ps1[:, ts(s, 512)],
                                     KT[hp][64:128, ts(tkc, 128)],
                                     QT[hp][64:128, :],
                                     start=True, stop=True,
                                     tile_position=(64, 0))
                # alibi add on DVE (psum + sbuf -> sbuf f32), then exp on ACT
                t0_ = sca_pool.tile([128, 1024], f32, tag="sca", name="sca0")
                nc.vector.tensor_tensor(t0_[:, :], ps0[:, :],
                                        als[0][:, ts(g, 1024)], OP.add)
                t1_ = sca_pool.tile([128, 1024], f32, tag="sca", name="sca1")
                nc.vector.tensor_tensor(t1_[:, :], ps1[:, :],
                                        als[1][:, ts(g, 1024)], OP.add)
                e0 = e_pool.tile([128, 1024], bf16, tag="expt", name="expt0")
                nc.scalar.activation(e0[:, :], t0_[:, :], AF.Exp)
                e1 = e_pool.tile([128, 1024], bf16, tag="expt", name="expt1")
                nc.scalar.activation(e1[:, :], t1_[:, :], AF.Exp)
                for s in range(2):
                    tkc = 2 * g + s
                    nc.tensor.matmul(av0[:, :],
                                     V[tkc][:, h0 * 65:h0 * 65 + 65],
                                     e0[:, ts(s, 512)],
                                     start=(tkc == 0), stop=(tkc == TC - 1))
                    nc.tensor.matmul(av1[:, :],
                                     V[tkc][:, h1 * 65:h1 * 65 + 65],
                                     e1[:, ts(s, 512)],
                                     start=(tkc == 0), stop=(tkc == TC - 1))
            for idx, av in ((0, av0), (1, av1)):
                rr = sm_pool.tile([1, 512], f32, tag="rr", name="rr")
                nc.vector.reciprocal(rr[:, :], av[64:65, :])
                r64 = sm_pool.tile([64, 512], f32, tag="r64", name="r64")
                nc.gpsimd.partition_broadcast(r64[:, :], rr[:, :])
                nc.vector.tensor_tensor(
                    yatt[hp][idx * 64:(idx + 1) * 64, :],
                    av[0:64, :], r64[:, :], OP.mult)

    # ---------------- attention out proj + residual --------------------
    y1 = []
    xo_scope = ExitStack()
    xo_pool = xo_scope.enter_context(tc.tile_pool(name="xo", bufs=CC))
    xo = []
    for c in range(CC):
        t = xo_pool.tile([128, TQ], f32, tag="xo", name="xo")
        nc.sync.dma_start(t[:, :], io["xT_own"][ts(c, 128), :])
        xo.append(t)
    with tc.tile_pool(name="woslab", bufs=3) as wo_pool, \
         tc.tile_pool(name="aops", bufs=2, space="PSUM") as ao_ps:
        for cc in range(CC):
            slab = wo_pool.tile([128, CC, 128], bf16, tag="woslab",
                                name="woslab")
            nc.sync.dma_start(
                slab[:, :, :],
                io["woT_t"][cc].rearrange("hd p m -> p hd m"))
            ps = ao_ps.tile([128, TQ], f32, tag="aops", name="aops")
            for hd in range(CC):
                nc.tensor.matmul(ps[:, :], slab[:, hd, :], yatt[hd][:, :],
                                 start=(hd == 0), stop=(hd == CC - 1))
            t = y1_pool.tile([128, TQ], f32, tag="y1", name="y1")
            nc.vector.tensor_tensor(t[:, :], ps[:, :], xo[cc][:, :], OP.add)
            y1.append(t)
    xo_scope.close()
    yatt_scope.close()
    al_scope.close()
    qkv_scope.close()   # free QT/KT/V

    if phases.startswith("att"):
        return

    # ---------------- rmsnorm #2 ---------------------------------------
    y2 = []
    with tc.tile_pool(name="rms2", bufs=4) as rms2_pool, \
         tc.tile_pool(name="rms2ps", bufs=1, space="PSUM") as rms2_ps:
        ssq2 = rms2_ps.tile([1, 512], f32, tag="ssq2", name="ssq2")
        for c in range(CC):
            sq2 = rms2_pool.tile([128, TQ], bf16, tag="sq2", name="sq2")
            nc.scalar.activation(sq2[:, :], y1[c][:, :], AF.Square)
            nc.tensor.matmul(ssq2[:, :], ones_col[:, :], sq2[:, :],
                             start=(c == 0), stop=(c == CC - 1))
        r2 = rms2_pool.tile([1, TQ], f32, tag="r2", name="r2")
        sd2 = rms2_pool.tile([1, TQ], f32, tag="sd2", name="sd2")
        nc.scalar.activation(sd2[:, :], ssq2[:, :], AF.Sqrt,
                             bias=0.0, scale=1.0 / C)
        nc.vector.reciprocal(r2[:, :], sd2[:, :])
        r2128 = rms2_pool.tile([128, TQ], f32, tag="r2b", name="r2b")
        nc.gpsimd.partition_broadcast(r2128[:, :], r2[:, :])
        for c in range(CC):
            t = y2_pool.tile([128, TQ], bf16, tag="y2", name="y2")
            nc.vector.tensor_tensor(t[:, :], y1[c][:, :], r2128[:, :], OP.mult)
            y2.append(t)

    # ---------------- MLP ----------------------------------------------
    with tc.tile_pool(name="hT", bufs=FC) as h_pool, \
         tc.tile_pool(name="wislab", bufs=3) as wi_pool, \
         tc.tile_pool(name="woslab2", bufs=3) as wo2_pool, \
         tc.tile_pool(name="mlpips", bufs=2, space="PSUM") as mi_ps, \
         tc.tile_pool(name="mlpops", bufs=4, space="PSUM") as mo_ps, \
         tc.tile_pool(name="mlpfin", bufs=4) as fin_pool:

        hT = []
        for half in range(2):
            ops = [mo_ps.tile([128, TQ], f32, tag="mops", name="mops")
                   for _ in range(4)]
            for f in range(FC):
                if half == 0:
                    slab = wi_pool.tile([128, CC, 128], bf16, tag="wislab",
                                        name="wislab")
                    nc.sync.dma_start(
                        slab[:, :, :],
                        io["w_inT_t"][f].rearrange("(cc p) m -> p cc m",
                                                   p=128))
                    ip = mi_ps.tile([128, TQ], f32, tag="mips", name="mips")
                    for c in range(CC):
                        nc.tensor.matmul(ip[:, :], slab[:, c, :], y2[c][:, :],
                                         start=(c == 0), stop=(c == CC - 1))
                    hf = h_pool.tile([128, TQ], bf16, tag="hT", name="hT")
                    nc.scalar.activation(hf[:, :], ip[:, :], AF.Gelu,
                                         bias=b_in_sb[:, f:f + 1])
                    hT.append(hf)
                oslab = wo2_pool.tile([128, 4, 128], bf16, tag="woslab2",
                                      name="woslab2")
                nc.sync.dma_start(
                    oslab[:, :, :],
                    io["w_outT_t"][f, 4 * half:4 * half + 4].rearrange(
                        "cc p m -> p cc m"))
                for i in range(4):
                    nc.tensor.matmul(ops[i][:, :], oslab[:, i, :], hT[f][:, :],
                                     start=(f == 0), stop=(f == FC - 1))
            for i in range(4):
                cc = 4 * half + i
                tmp = fin_pool.tile([128, TQ], f32, tag="fin", name="fin")
                nc.vector.tensor_tensor(tmp[:, :], ops[i][:, :], y1[cc][:, :],
                                        OP.add)
                out_sb = fin_pool.tile([128, TQ], f32, tag="fin2", name="fin2")
                nc.vector.tensor_scalar_add(out_sb[:, :], tmp[:, :],
                                            b_out_sb[:, cc:cc + 1])
                nc.sync.dma_start(io["outT"][ts(cc, 128), :], out_sb[:, :])


_NC_CACHE = {}


def _build_nc(repeats=1, phases="all"):
    key = (repeats, phases)
    if key in _NC_CACHE:
        return _NC_CACHE[key]
    from contextlib import ExitStack as _ES
    from concourse import bacc
    import concourse.tile as tile
    import concourse.mybir as mybir

    dt = mybir.dt
    nc = bacc.Bacc("TRN2", target_bir_lowering=False, debug=False,
                   num_devices=NCORES)

    io = {}
    spec = dict(
        xT_own=((C, TQ), dt.float32), xbf=((C, T), dt.bfloat16),
        xbf_own=((C, TQ), dt.bfloat16),
        alibi_t=((H, T, TQ), dt.bfloat16),
        cs2T=((128, T), dt.bfloat16), ss2T=((128, T), dt.bfloat16),
        cs2T_own=((128, TQ), dt.bfloat16), ss2T_own=((128, TQ), dt.bfloat16),
        wqkT_t=((16, C, 128), dt.bfloat16), wvT_r=((CC, 128, HD), dt.bfloat16),
        woT_t=((CC, CC, 128, 128), dt.bfloat16),
        w_inT_t=((FC, C, 128), dt.bfloat16),
        w_outT_t=((FC, CC, 128, 128), dt.bfloat16),
        b_in_t=((128, FC), dt.float32), b_out_t=((128, CC), dt.float32),
    )
    for name, (shape, d) in spec.items():
        io[name] = nc.dram_tensor(name, list(shape), d, kind="ExternalInput").ap()
    io["outT"] = nc.dram_tensor("outT", [C, TQ], dt.float32,
                                kind="ExternalOutput").ap()

    with tile.TileContext(nc, pool_alloc_mode="queue") as tc:
        for rep in range(repeats):
            if rep:
                tc.strict_bb_all_engine_barrier()
            with ExitStack() as ctx:
                build(nc, tc, io, ctx, phases=phases)
    nc.compile()
    _NC_CACHE[key] = nc
    return nc


def _run(in_maps, trace):
    global LAST_RESULTS
    import concourse.bass_utils as bass_utils
    nc = _build_nc()
    results = bass_utils.run_bass_kernel_spmd(
        nc, in_maps, core_ids=list(range(NCORES)), trace=trace)
    LAST_RESULTS = results
    return results




def bench(in_maps, iters=3, reps=4, phases="all"):
    """Marginal per-kernel device time: build two NEFFs (1x body, `iters`x
    body with all-engine barriers between repeats), run both with
    device-resident inputs, report (T_iters - T_1)/(iters-1)."""
    import time
    import jax
    from jax.sharding import Mesh, PartitionSpec
    from jax.experimental.shard_map import shard_map
    import concourse.mybir as mybir
    from concourse import bass2jax
    from concourse.bass2jax import _bass_exec_p, install_neuronx_cc_hook

    install_neuronx_cc_hook()
    timings = {}
    for n_rep in (1, iters):
        nc = _build_nc(n_rep, phases)
        in_names, out_names, out_avals, zero_outs = [], [], [], []
        partition_name = (nc.partition_id_tensor.name
                          if nc.partition_id_tensor else None)
        for alloc in nc.m.functions[0].allocations:
            if not isinstance(alloc, mybir.MemoryLocationSet):
                continue
            name = alloc.memorylocations[0].name
            if alloc.kind == "ExternalInput":
                if name != partition_name:
                    in_names.append(name)
            elif alloc.kind == "ExternalOutput":
                shape = tuple(alloc.tensor_shape)
                dtype = mybir.dt.np(alloc.dtype)
                out_names.append(name)
                out_avals.append(jax.core.ShapedArray(shape, dtype))
                zero_outs.append(np.zeros(shape, dtype))
        n_params = len(in_names)
        n_outs = len(out_avals)
        all_in_names = list(in_names) + list(out_names)
        if partition_name is not None:
            all_in_names.append(partition_name)

        def _body(*args, _nc=nc, _avals=tuple(out_avals),
                  _innames=tuple(all_in_names), _outnames=tuple(out_names),
                  _pname=partition_name):
            operands = list(args)
            if _pname is not None:
                operands.append(bass2jax.partition_id_tensor())
            outs = _bass_exec_p.bind(
                *operands, out_avals=_avals, in_names=_innames,
                out_names=_outnames, lowering_input_output_aliases=(),
                sim_require_finite=True, sim_require_nnan=True, nc=_nc)
            return tuple(outs)

        devices = jax.devices()[:NCORES]
        mesh = Mesh(np.asarray(devices), ("core",))
        in_specs = (PartitionSpec("core"),) * (n_params + n_outs)
        out_specs = (PartitionSpec("core"),) * n_outs
        per_core = [[np.asarray(m[name]) for name in in_names]
                    for m in in_maps]
        concat_in = [np.concatenate([per_core[c][i] for c in range(NCORES)],
                                    axis=0) for i in range(n_params)]
        concat_zeros = [np.zeros((NCORES * z.shape[0], *z.shape[1:]), z.dtype)
                        for z in zero_outs]
        dev_in = [jax.device_put(a) for a in concat_in]

        donate = tuple(range(n_params, n_params + n_outs))
        fn = jax.jit(shard_map(_body, mesh=mesh, in_specs=in_specs,
                               out_specs=out_specs, check_rep=False),
                     donate_argnums=donate, keep_unused=True)
        samples = []
        for i in range(reps + 1):
            # fresh, value-varying zero buffers each call defeat any
            # result-memoization in the execution path
            zs = [np.full((NCORES * z.shape[0], *z.shape[1:]), 1e-6 * i,
                          z.dtype) for z in zero_outs]
            t0 = time.perf_counter()
            outs = fn(*dev_in, *zs)
            jax.block_until_ready(outs)
            dt = time.perf_counter() - t0
            if i > 0:
                samples.append(dt)
        timings[n_rep] = min(samples)
    per_iter = (timings[iters] - timings[1]) / (iters - 1)
    return per_iter * 1e9, timings[1] * 1e9


def kernel(**inputs):
    hp = host_prep(inputs)
    in_maps = []
    for core in range(NCORES):
        ci = core_inputs(hp, core)
        in_maps.append({k: np.ascontiguousarray(v) for k, v in ci.items()})
    trace = bool(int(os.environ.get("KERNEL_TRACE", "0")))
    results = _run(in_maps, trace)
    out = np.zeros((B, T, C), np.float32)
    for core in range(NCORES):
        b, j = core // G, core % G
        out[b, j * TQ:(j + 1) * TQ, :] = results.results[core]["outT"].T
    return out


if __name__ == "__main__":
    import reference
    inputs = reference.setup_inputs()
    out = kernel(**{k: np.asarray(v) for k, v in inputs.items()})
    exp = np.asarray(reference.reference(**inputs))
    err = np.abs(out - exp).max() / np.abs(exp).max()
    print("rel(absmax) err:", err)



# revision 11
# speedup vs baseline: 1.3383x; 1.3383x over previous
"""Trainium2 Bass kernel for nn_AttentionBlock (B=2,T=2048,C=1024,H=16,D=64,F=4096).

Sharding: 8 cores = 2 batches x 4 query-chunks of 512 tokens. Each core
computes K/V for its own 512 tokens, AllGathers K/V within the 4-core
batch group, then runs attention + MLP for its own 512 query tokens.
All matmuls run in bf16 with f32 PSUM accumulation; the residual stream,
softmax denominators and rms statistics stay in f32.

v2 changes vs baseline:
 - host precomputes exp(alibi) in a head-pair-interleaved contiguous
   layout; the device multiplies exp(scores) by it (bf16 DVE 2x) instead
   of adding alibi into f32 scores (slow DVE 1x op).
 - all weight slabs / x / alibi are pre-transposed on host so every DMA
   is contiguous per partition line (>=1KB descriptors, near line rate).
 - attention inner loop packs both heads of a pair into one [128,1024]
   PSUM tile per key-chunk (scps bufs=3 for PE/ACT overlap), AV pair in
   one 2-bank PSUM tile.
 - rope moved to ACT-copy + bf16 DVE ops; fused final bias+residual adds.
"""
import os
from contextlib import ExitStack

import numpy as np
import ml_dtypes

BF16 = ml_dtypes.bfloat16

B, T, C, H, D, F = 2, 2048, 1024, 16, 64, 4096
NCORES, G = 8, 4
TQ = T // G                 # 512 query tokens per core
HD = H * D                  # 1024
EPS = 1e-8
CC = C // 128               # 8 channel chunks
TC = T // 128               # 16 token chunks
FC = F // 128               # 32 hidden chunks
HP = H // 2                 # 8 head pairs

# stream_shuffle permutes lanes within each 32-partition quadrant
# (out[s*32+i] = in[s*32+mask[i]]). We host-permute the head dim so rope
# partners (d, d+32) sit on adjacent partitions; the swap is then [1,0,3,2,..].
ROPE_PERM = np.arange(64).reshape(2, 32).T.reshape(-1)   # [0,32,1,33,...]
SWAP_MASK = [i ^ 1 for i in range(32)]

LAST_RESULTS = None  # BassKernelResults of the last run (for test.py)


def host_prep(inputs):
    x = np.asarray(inputs["x"], np.float32)
    alibi = np.asarray(inputs["alibi"], np.float32)
    rot = np.asarray(inputs["rotational"], np.float32)
    g_att = np.asarray(inputs["g_att"], np.float32)
    g_mlp = np.asarray(inputs["g_mlp"], np.float32)
    w_qkv = np.asarray(inputs["w_qkv"], np.float32)
    w_att_out = np.asarray(inputs["w_att_out"], np.float32)
    w_mlp_in = np.asarray(inputs["w_mlp_in"], np.float32)
    b_mlp_in = np.asarray(inputs["b_mlp_in"], np.float32)
    w_mlp_out = np.asarray(inputs["w_mlp_out"], np.float32)
    b_mlp_out = np.asarray(inputs["b_mlp_out"], np.float32)

    wg = w_qkv * g_att[None, :]                   # fold g_att into qkv
    wg[:HD] *= 1.0 / np.sqrt(np.float32(D))       # fold attn scale into w_q

    # permute each head's 64 q/k output dims so rope pairs are adjacent
    wqk_p = wg[:2 * HD].reshape(2, H, 64, C)[:, :, ROPE_PERM, :].reshape(
        2 * HD, C)
    wqkvT = np.ascontiguousarray(
        np.concatenate([wqk_p, wg[2 * HD:]], 0).T).astype(BF16)    # [C, 3HD]
    # Q/K slabs, contiguous per partition: slab[e, p, cc*128+m]
    #   = wqkvT[cc*128+p, e*128+m]
    wqk_slabs = np.ascontiguousarray(
        wqkvT[:, :2 * HD].reshape(CC, 128, 16, 128)
        .transpose(2, 1, 0, 3).reshape(16, 128, C))                # [16,128,C]
    wvT_r = np.ascontiguousarray(
        wqkvT[:, 2 * HD:].reshape(CC, 128, HD))                    # [8,128,HD]
    woT = np.ascontiguousarray(w_att_out.T).astype(BF16)           # [HD, C]
    wo_slabs = np.ascontiguousarray(
        woT.reshape(CC, 128, CC, 128).transpose(2, 1, 0, 3)
        .reshape(CC, 128, C))                                      # [8,128,C]
    w_inT = np.ascontiguousarray((w_mlp_in * g_mlp[None, :]).T).astype(BF16)
    w_in_slabs = np.ascontiguousarray(
        w_inT.reshape(CC, 128, FC, 128).transpose(2, 1, 0, 3)
        .reshape(FC, 128, C))                                      # [32,128,C]
    w_outT = np.ascontiguousarray(w_mlp_out.T).astype(BF16)        # [F, C]
    w_out_slabs = w_outT.reshape(FC, 128, C)                       # [32,128,C]

    b_in_t = np.ascontiguousarray(b_mlp_in.reshape(FC, 128).T)     # [128, 32]
    b_out_t = np.ascontiguousarray(b_mlp_out.reshape(CC, 128).T)   # [128, 8]

    cosT = np.cos(rot).T.astype(np.float32)                        # [D, T]
    sinT = np.sin(rot).T.astype(np.float32)
    sgn = np.where(np.arange(D) < D // 2, -1.0, 1.0).astype(np.float32)
    ssinT = sinT * sgn[:, None]
    cosT = cosT[ROPE_PERM]                             # match head-dim perm
    # pre-swap the sign-sin rows: device computes swap(x * ss) == swap(x) * s
    ssinT = ssinT[ROPE_PERM][np.arange(64) ^ 1]
    cs2T = np.ascontiguousarray(np.tile(cosT, (2, 1))).astype(BF16)  # [128, T]
    ss2T = np.ascontiguousarray(np.tile(ssinT, (2, 1))).astype(BF16)

    exp_alibi = np.exp(alibi)                          # [H, T, T] f32
    xT = {b: np.ascontiguousarray(x[b].T) for b in range(B)}       # [C,T] f32
    return dict(exp_alibi=exp_alibi, wqk_slabs=wqk_slabs, wvT_r=wvT_r,
                wo_slabs=wo_slabs, w_in_slabs=w_in_slabs,
                w_out_slabs=w_out_slabs, b_in_t=b_in_t, b_out_t=b_out_t,
                cs2T=cs2T, ss2T=ss2T, xT=xT)


def core_inputs(hp, core):
    b, j = core // G, core % G
    q0 = j * TQ
    # exp(alibi) in [hp, p(key%128), tkc, h01, q] layout, contiguous
    al = hp["exp_alibi"][:, q0:q0 + TQ, :]            # [H, TQ, T]
    alT = np.ascontiguousarray(al.transpose(0, 2, 1))  # [H, T, TQ]
    ealibi = np.ascontiguousarray(
        alT.reshape(HP, 2, TC, 128, TQ).transpose(0, 3, 2, 1, 4)
        .reshape(HP, 128, TC * 2 * TQ)).astype(BF16)   # [8, 128, 16384]
    xo = np.ascontiguousarray(hp["xT"][b][:, q0:q0 + TQ])          # f32 [C,TQ]
    xon = np.ascontiguousarray(
        xo.reshape(CC, 128, TQ).transpose(1, 0, 2)
        .reshape(128, CC * TQ)).astype(BF16)                       # [128,4096]
    return dict(
        xT_own=xo, xon=xon, ealibi=ealibi,
        cs2T_own=np.ascontiguousarray(hp["cs2T"][:, q0:q0 + TQ]),
        ss2T_own=np.ascontiguousarray(hp["ss2T"][:, q0:q0 + TQ]),
        wqk_slabs=hp["wqk_slabs"], wvT_r=hp["wvT_r"],
        wo_slabs=hp["wo_slabs"], w_in_slabs=hp["w_in_slabs"],
        w_out_slabs=hp["w_out_slabs"],
        b_in_t=hp["b_in_t"], b_out_t=hp["b_out_t"],
    )


def build(nc, tc, io, ctx, phases="all"):
    import concourse.bass as bass
    import concourse.mybir as mybir
    from concourse.bass import ts

    dt = mybir.dt
    AF = mybir.ActivationFunctionType
    OP = mybir.AluOpType
    f32, bf16 = dt.float32, dt.bfloat16

    def pool(name, bufs, space="SBUF"):
        return ctx.enter_context(tc.tile_pool(name=name, bufs=bufs, space=space))

    consts = pool("consts", 1)
    ones_col = consts.tile([128, 1], bf16, tag="ones", name="ones")
    nc.vector.memset(ones_col[:, :], 1.0)
    ones_f32 = consts.tile([128, 1], f32, tag="ones32", name="ones32")
    nc.vector.memset(ones_f32[:, :], 1.0)
    b_in_sb = consts.tile([128, FC], f32, tag="b_in", name="b_in")
    nc.sync.dma_start(b_in_sb[:, :], io["b_in_t"][:, :])
    b_out_sb = consts.tile([128, CC], f32, tag="b_out", name="b_out")
    nc.sync.dma_start(b_out_sb[:, :], io["b_out_t"][:, :])

    y1_pool = pool("y1", CC)

    qkv_scope = ExitStack()

    def qpool(name, bufs, space="SBUF"):
        return qkv_scope.enter_context(
            tc.tile_pool(name=name, bufs=bufs, space=space))

    QT_pool = qpool("QT", CC)
    KT_pool = qpool("KT", CC)
    V_pool = qpool("V", TC)
    QT, KT, V = [], [], []

    with ExitStack() as p1:
        def ppool(name, bufs, space="SBUF"):
            return p1.enter_context(
                tc.tile_pool(name=name, bufs=bufs, space=space))

        xon = ppool("xon", 1).tile([128, CC * TQ], bf16, tag="xon", name="xon")
        nc.sync.dma_start(xon[:, :], io["xon"][:, :])
        xon3 = xon.rearrange("p (cc q) -> p cc q", q=TQ)

        csr_pool = ppool("csr", 1)
        cso_r = csr_pool.tile([128, TQ], bf16, tag="csor", name="csor")
        sso_r = csr_pool.tile([128, TQ], bf16, tag="ssor", name="ssor")
        ro_col = csr_pool.tile([128, TQ // 128], f32, tag="rocol", name="rocol")

        stats_scope = ExitStack()
        rms1_pool = stats_scope.enter_context(tc.tile_pool(name="rms1", bufs=1))
        sq_pool = stats_scope.enter_context(tc.tile_pool(name="sq", bufs=2))
        rms1_ps = stats_scope.enter_context(
            tc.tile_pool(name="rms1ps", bufs=1, space="PSUM"))

        # own-token rmsnorm stats
        ssqo = rms1_ps.tile([1, 512], f32, tag="ssqo", name="ssqo")
        for c in range(CC):
            sqo = sq_pool.tile([128, TQ], bf16, tag="sqo", name="sqo")
            nc.scalar.activation(sqo[:, :], xon3[:, c, :], AF.Square)
            nc.tensor.matmul(ssqo[:, :], ones_col[:, :], sqo[:, :],
                             start=(c == 0), stop=(c == CC - 1))
        ro_sb = rms1_pool.tile([1, TQ], f32, tag="ro", name="ro")
        sdo_sb = rms1_pool.tile([1, TQ], f32, tag="sdo", name="sdo")
        nc.scalar.activation(sdo_sb[:, :], ssqo[:, :], AF.Sqrt,
                             bias=0.0, scale=1.0 / C)
        nc.vector.reciprocal(ro_sb[:, :], sdo_sb[:, :])
        ro128 = rms1_pool.tile([128, TQ], f32, tag="rob", name="rob")
        nc.gpsimd.partition_broadcast(ro128[:, :], ro_sb[:, :])

        # rope tables with r folded in
        with tc.tile_pool(name="cstmp", bufs=1) as cst:
            for dst, src_name in ((cso_r, "cs2T_own"), (sso_r, "ss2T_own")):
                tmp = cst.tile([128, TQ], bf16, tag="cstmp", name="cstmp")
                nc.sync.dma_start(tmp[:, :], io[src_name][:, :])
                nc.vector.tensor_tensor(dst[:, :], tmp[:, :], ro128[:, :],
                                        OP.mult)

        # transpose r -> per-token-chunk columns for the V scaling
        rt_ps = stats_scope.enter_context(
            tc.tile_pool(name="rtps", bufs=2, space="PSUM"))
        for t in range(TQ // 128):
            ps = rt_ps.tile([128, 1], f32, tag="rtps", name="rtps")
            nc.tensor.matmul(ps[:, :], ro_sb[0:1, ts(t, 128)],
                             ones_f32[0:1, 0:1], start=True, stop=True)
            nc.vector.tensor_copy(ro_col[:, t:t + 1], ps[:, :])
        stats_scope.close()

        # ---------------- QKV projections + rope -----------------------
        wslab_pool = ppool("wslab", 3)
        qk_ps = ppool("qkps", 3, "PSUM")
        rope_pool = ppool("ropet", 3)
        kv_pool = ppool("kvout", 3)

        def rope(dst_ap, ps):
            pbf = rope_pool.tile([128, 512], bf16, tag="ropeP", name="ropeP")
            nc.scalar.copy(pbf[:, :], ps[:, :])
            qc = rope_pool.tile([128, 512], bf16, tag="ropeA", name="ropeA")
            nc.vector.tensor_tensor(qc[:, :], pbf[:, :], cso_r[:, :], OP.mult)
            tmp = rope_pool.tile([128, 512], bf16, tag="ropeB", name="ropeB")
            nc.vector.tensor_tensor(tmp[:, :], pbf[:, :], sso_r[:, :], OP.mult)
            qs = rope_pool.tile([128, 512], bf16, tag="ropeC", name="ropeC")
            nc.vector.stream_shuffle(qs[:, :], tmp[:, :], SWAP_MASK)
            nc.vector.tensor_tensor(dst_ap, qc[:, :], qs[:, :], OP.add)

        NV = TQ // 128           # 4 own V chunks
        KO_ELEMS = C * TQ        # 8 chunks of [128, 512]
        VO_ELEMS = NV * 128 * 1040
        dram_pool = qpool("kvdram", 1, "DRAM")
        kv_own = dram_pool.tile([KO_ELEMS + VO_ELEMS], bf16,
                                tag="kvown", name="kvown")
        kv_gath = dram_pool.tile([G * (KO_ELEMS + VO_ELEMS)], bf16,
                                 tag="kvgath", name="kvgath")
        for e in range(CC):      # K (own tokens), roped, to DRAM
            slab = wslab_pool.tile([128, C], bf16, tag="wslab", name="wslab")
            nc.sync.dma_start(slab[:, :], io["wqk_slabs"][CC + e][:, :])
            slab3 = slab.rearrange("p (cc m) -> p cc m", m=128)
            ps = qk_ps.tile([128, 512], f32, tag="qkps", name="qkps")
            for c in range(CC):
                nc.tensor.matmul(ps[:, :], slab3[:, c, :], xon3[:, c, :],
                                 start=(c == 0), stop=(c == CC - 1))
            ko = kv_pool.tile([128, TQ], bf16, tag="ko", name="ko")
            rope(ko[:, :], ps)
            nc.sync.dma_start(
                kv_own[e * 128 * TQ:(e + 1) * 128 * TQ].rearrange(
                    "(p q) -> p q", q=TQ), ko[:, :])
        # V (own tokens), padded per-head layout
        wv_pool = ppool("wvr", CC)
        wv = []
        for c in range(CC):
            t = wv_pool.tile([128, HD], bf16, tag="wvr", name="wvr")
            nc.sync.dma_start(t[:, :], io["wvT_r"][c][:, :])
            wv.append(t)
        for tcn in range(NV):
            vt = kv_pool.tile([128, 16 * 65], bf16, tag="vo", name="vo")
            vt3 = vt.rearrange("p (h x) -> p h x", x=65)
            nc.vector.memset(vt3[:, :, 64:65], 1.0)
            for n in range(2):
                ps = qk_ps.tile([128, 512], f32, tag="qkps", name="qkps")
                for c in range(CC):
                    nc.tensor.matmul(ps[:, :], xon3[:, c, ts(tcn, 128)],
                                     wv[c][:, ts(n, 512)],
                                     start=(c == 0), stop=(c == CC - 1))
                nc.vector.tensor_scalar_mul(
                    vt3[:, ts(n, 8), 0:64],
                    ps.rearrange("p (h d) -> p h d", d=64),
                    ro_col[:, tcn:tcn + 1])
            nc.sync.dma_start(
                kv_own[KO_ELEMS + tcn * 128 * 1040:
                       KO_ELEMS + (tcn + 1) * 128 * 1040].rearrange(
                    "(p q) -> p q", q=1040), vt[:, :])
        for e in range(CC):      # Q (own tokens) — overlaps the gather
            slab = wslab_pool.tile([128, C], bf16, tag="wslab", name="wslab")
            nc.sync.dma_start(slab[:, :], io["wqk_slabs"][e][:, :])
            slab3 = slab.rearrange("p (cc m) -> p cc m", m=128)
            ps = qk_ps.tile([128, 512], f32, tag="qkps", name="qkps")
            for c in range(CC):
                nc.tensor.matmul(ps[:, :], slab3[:, c, :], xon3[:, c, :],
                                 start=(c == 0), stop=(c == CC - 1))
            qt = QT_pool.tile([128, TQ], bf16, tag="QT", name="QT")
            rope(qt[:, :], ps)
            QT.append(qt)
        # gather within the 4-core batch group
        nc.gpsimd.collective_compute(
            "AllGather", mybir.AluOpType.bypass,
            replica_groups=[[0, 1, 2, 3], [4, 5, 6, 7]],
            ins=[kv_own[:].opt()], outs=[kv_gath[:].opt()])

    RK = KO_ELEMS + VO_ELEMS
    for e in range(CC):      # assemble KT [128, T]
        kt = KT_pool.tile([128, T], bf16, tag="KT", name="KT")
        for r in range(G):
            nc.sync.dma_start(
                kt[:, r * TQ:(r + 1) * TQ],
                kv_gath[r * RK + e * 128 * TQ:
                        r * RK + (e + 1) * 128 * TQ].rearrange(
                    "(p q) -> p q", q=TQ))
        KT.append(kt)
    for tkc in range(TC):    # assemble V tiles [128, 1040]
        r, sub = tkc // NV, tkc % NV
        vt = V_pool.tile([128, 16 * 65], bf16, tag="V", name="V")
        nc.sync.dma_start(
            vt[:, :],
            kv_gath[r * RK + KO_ELEMS + sub * 128 * 1040:
                    r * RK + KO_ELEMS + (sub + 1) * 128 * 1040
                    ].rearrange("(p q) -> p q", q=1040))
        V.append(vt)

    if phases == "qkv":
        qkv_scope.close()
        return

    # ---------------- attention ---------------------------------------
    al_scope = ExitStack()
    al_pool = al_scope.enter_context(tc.tile_pool(name="ealibi", bufs=3))

    def load_ealibi(hpair, half):
        # half a head-pair slab: tkc chunks [8*half, 8*half+8)
        al = al_pool.tile([128, TC * TQ], bf16, tag="ealibi", name="ealibi")
        for dd in range(2):
            nc.sync.dma_start(
                al[:, ts(dd, 4096)],
                io["ealibi"][hpair][:, half * 8192 + dd * 4096:
                                    half * 8192 + (dd + 1) * 4096])
        return al

    al_cache = {(0, 0): load_ealibi(0, 0), (0, 1): load_ealibi(0, 1),
                (1, 0): load_ealibi(1, 0)}

    yatt_scope = ExitStack()
    yatt_pool = yatt_scope.enter_context(tc.tile_pool(name="yatt", bufs=CC))
    yatt = [yatt_pool.tile([128, TQ], bf16, tag="yatt", name="yatt")
            for _ in range(CC)]

    with tc.tile_pool(name="scps", bufs=3, space="PSUM") as sc_ps, \
         tc.tile_pool(name="avps", bufs=1, space="PSUM") as av_ps, \
         tc.tile_pool(name="expt", bufs=3) as e_pool, \
         tc.tile_pool(name="emul", bufs=3) as em_pool, \
         tc.tile_pool(name="attsm", bufs=1) as sm_pool:

        def get_ealibi(hpair, half):
            al = al_cache.pop((hpair, half), None)
            if al is None:
                al = load_ealibi(hpair, half)
            want = (hpair, half + 1) if half == 0 else (hpair + 1, 0)
            if want[0] < HP and want not in al_cache:
                al_cache[want] = load_ealibi(*want)
            return al

        for hpair in range(HP):
            h0, h1 = 2 * hpair, 2 * hpair + 1
            av = av_ps.tile([65, 1024], f32, tag="av", name="av")
            for tkc in range(TC):
                if tkc % 8 == 0:
                    al = get_ealibi(hpair, tkc // 8)
                ps = sc_ps.tile([128, 1024], f32, tag="scps", name="scps")
                nc.tensor.matmul(ps[:, 0:512],
                                 KT[hpair][0:64, ts(tkc, 128)],
                                 QT[hpair][0:64, :],
                                 start=True, stop=True,
                                 tile_position=(0, 0))
                nc.tensor.matmul(ps[:, 512:1024],
                                 KT[hpair][64:128, ts(tkc, 128)],
                                 QT[hpair][64:128, :],
                                 start=True, stop=True,
                                 tile_position=(64, 0))
                et = e_pool.tile([128, 1024], bf16, tag="expt", name="expt")
                nc.scalar.activation(et[:, :], ps[:, :], AF.Exp)
                em = em_pool.tile([128, 1024], bf16, tag="emul", name="emul")
                nc.vector.tensor_tensor(em[:, :], et[:, :],
                                        al[:, ts(tkc % 8, 1024)], OP.mult)
                nc.tensor.matmul(av[:, 0:512],
                                 V[tkc][:, h0 * 65:h0 * 65 + 65],
                                 em[:, 0:512],
                                 start=(tkc == 0), stop=(tkc == TC - 1))
                nc.tensor.matmul(av[:, 512:1024],
                                 V[tkc][:, h1 * 65:h1 * 65 + 65],
                                 em[:, 512:1024],
                                 start=(tkc == 0), stop=(tkc == TC - 1))
            rr = sm_pool.tile([1, 1024], f32, tag="rr", name="rr")
            nc.vector.reciprocal(rr[:, :], av[64:65, :])
            r64 = sm_pool.tile([64, 1024], f32, tag="r64", name="r64")
            nc.gpsimd.partition_broadcast(r64[:, :], rr[:, :])
            nc.vector.tensor_tensor(yatt[hpair][0:64, :],
                                    av[0:64, 0:512], r64[:, 0:512], OP.mult)
            nc.vector.tensor_tensor(yatt[hpair][64:128, :],
                                    av[0:64, 512:1024], r64[:, 512:1024],
                                    OP.mult)

    # ---------------- attention out proj + residual --------------------
    y1 = []
    xo_scope = ExitStack()
    xo_pool = xo_scope.enter_context(tc.tile_pool(name="xo", bufs=CC))
    xo = []
    for c in range(CC):
        t = xo_pool.tile([128, TQ], f32, tag="xo", name="xo")
        nc.sync.dma_start(t[:, :], io["xT_own"][ts(c, 128), :])
        xo.append(t)
    with tc.tile_pool(name="woslab", bufs=3) as wo_pool, \
         tc.tile_pool(name="aops", bufs=2, space="PSUM") as ao_ps:
        for cc in range(CC):
            slab = wo_pool.tile([128, C], bf16, tag="woslab", name="woslab")
            nc.sync.dma_start(slab[:, :], io["wo_slabs"][cc][:, :])
            slab3 = slab.rearrange("p (hd m) -> p hd m", m=128)
            ps = ao_ps.tile([128, TQ], f32, tag="aops", name="aops")
            for hd in range(CC):
                nc.tensor.matmul(ps[:, :], slab3[:, hd, :], yatt[hd][:, :],
                                 start=(hd == 0), stop=(hd == CC - 1))
            t = y1_pool.tile([128, TQ], f32, tag="y1", name="y1")
            nc.vector.tensor_tensor(t[:, :], ps[:, :], xo[cc][:, :], OP.add)
            y1.append(t)
    xo_scope.close()
    yatt_scope.close()
    al_scope.close()
    qkv_scope.close()   # free QT/KT/V

    if phases.startswith("att"):
        return

    # ---------------- rmsnorm #2 ---------------------------------------
    y2_pool = pool("y2", CC)
    y2 = []
    with tc.tile_pool(name="rms2", bufs=4) as rms2_pool, \
         tc.tile_pool(name="rms2ps", bufs=1, space="PSUM") as rms2_ps:
        ssq2 = rms2_ps.tile([1, 512], f32, tag="ssq2", name="ssq2")
        for c in range(CC):
            sq2 = rms2_pool.tile([128, TQ], bf16, tag="sq2", name="sq2")
            nc.scalar.activation(sq2[:, :], y1[c][:, :], AF.Square)
            nc.tensor.matmul(ssq2[:, :], ones_col[:, :], sq2[:, :],
                             start=(c == 0), stop=(c == CC - 1))
        r2 = rms2_pool.tile([1, TQ], f32, tag="r2", name="r2")
        sd2 = rms2_pool.tile([1, TQ], f32, tag="sd2", name="sd2")
        nc.scalar.activation(sd2[:, :], ssq2[:, :], AF.Sqrt,
                             bias=0.0, scale=1.0 / C)
        nc.vector.reciprocal(r2[:, :], sd2[:, :])
        r2128 = rms2_pool.tile([128, TQ], f32, tag="r2b", name="r2b")
        nc.gpsimd.partition_broadcast(r2128[:, :], r2[:, :])
        for c in range(CC):
            t = y2_pool.tile([128, TQ], bf16, tag="y2", name="y2")
            nc.vector.tensor_tensor(t[:, :], y1[c][:, :], r2128[:, :], OP.mult)
            y2.append(t)

    # ---------------- MLP ----------------------------------------------
    with tc.tile_pool(name="hT", bufs=FC) as h_pool, \
         tc.tile_pool(name="wislab", bufs=3) as wi_pool, \
         tc.tile_pool(name="woslab2", bufs=3) as wo2_pool, \
         tc.tile_pool(name="mlpips", bufs=2, space="PSUM") as mi_ps, \
         tc.tile_pool(name="mlpops", bufs=4, space="PSUM") as mo_ps, \
         tc.tile_pool(name="mlpfin", bufs=4) as fin_pool:

        hT = []
        for half in range(2):
            ops = [mo_ps.tile([128, TQ], f32, tag="mops", name="mops")
                   for _ in range(4)]
            for f in range(FC):
                if half == 0:
                    slab = wi_pool.tile([128, C], bf16, tag="wislab",
                                        name="wislab")
                    nc.sync.dma_start(slab[:, :], io["w_in_slabs"][f][:, :])
                    slab3 = slab.rearrange("p (cc m) -> p cc m", m=128)
                    ip = mi_ps.tile([128, TQ], f32, tag="mips", name="mips")
                    for c in range(CC):
                        nc.tensor.matmul(ip[:, :], slab3[:, c, :], y2[c][:, :],
                                         start=(c == 0), stop=(c == CC - 1))
                    hf = h_pool.tile([128, TQ], bf16, tag="hT", name="hT")
                    nc.scalar.activation(hf[:, :], ip[:, :], AF.Gelu,
                                         bias=b_in_sb[:, f:f + 1])
                    hT.append(hf)
                oslab = wo2_pool.tile([128, 512], bf16, tag="woslab2",
                                      name="woslab2")
                nc.sync.dma_start(
                    oslab[:, :],
                    io["w_out_slabs"][f][:, ts(half, 512)])
                oslab3 = oslab.rearrange("p (i m) -> p i m", m=128)
                for i in range(4):
                    nc.tensor.matmul(ops[i][:, :], oslab3[:, i, :],
                                     hT[f][:, :],
                                     start=(f == 0), stop=(f == FC - 1))
            for i in range(4):
                cc = 4 * half + i
                out_sb = fin_pool.tile([128, TQ], f32, tag="fin", name="fin")
                nc.vector.scalar_tensor_tensor(
                    out_sb[:, :], ops[i][:, :], b_out_sb[:, cc:cc + 1],
                    y1[cc][:, :], OP.add, OP.add)
                nc.sync.dma_start(io["outT"][ts(cc, 128), :], out_sb[:, :])


_NC_CACHE = {}


def _build_nc(repeats=1, phases="all"):
    key = (repeats, phases)
    if key in _NC_CACHE:
        return _NC_CACHE[key]
    from contextlib import ExitStack as _ES
    from concourse import bacc
    import concourse.tile as tile
    import concourse.mybir as mybir

    dt = mybir.dt
    nc = bacc.Bacc("TRN2", target_bir_lowering=False, debug=False,
                   num_devices=NCORES)

    io = {}
    spec = dict(
        xT_own=((C, TQ), dt.float32),
        xon=((128, CC * TQ), dt.bfloat16),
        ealibi=((HP, 128, TC * 2 * TQ), dt.bfloat16),
        cs2T_own=((128, TQ), dt.bfloat16), ss2T_own=((128, TQ), dt.bfloat16),
        wqk_slabs=((16, 128, C), dt.bfloat16),
        wvT_r=((CC, 128, HD), dt.bfloat16),
        wo_slabs=((CC, 128, C), dt.bfloat16),
        w_in_slabs=((FC, 128, C), dt.bfloat16),
        w_out_slabs=((FC, 128, C), dt.bfloat16),
        b_in_t=((128, FC), dt.float32), b_out_t=((128, CC), dt.float32),
    )
    for name, (shape, d) in spec.items():
        io[name] = nc.dram_tensor(name, list(shape), d, kind="ExternalInput").ap()
    io["outT"] = nc.dram_tensor("outT", [C, TQ], dt.float32,
                                kind="ExternalOutput").ap()

    with tile.TileContext(nc, pool_alloc_mode="queue") as tc:
        for rep in range(repeats):
            if rep:
                tc.strict_bb_all_engine_barrier()
            with ExitStack() as ctx:
                build(nc, tc, io, ctx, phases=phases)
    nc.compile()
    _NC_CACHE[key] = nc
    return nc


def _run(in_maps, trace):
    global LAST_RESULTS
    import concourse.bass_utils as bass_utils
    nc = _build_nc()
    results = bass_utils.run_bass_kernel_spmd(
        nc, in_maps, core_ids=list(range(NCORES)), trace=trace)
    LAST_RESULTS = results
    return results


def bench(in_maps, iters=3, reps=4, phases="all"):
    """Marginal per-kernel device time: build two NEFFs (1x body, `iters`x
    body with all-engine barriers between repeats), run both with
    device-resident inputs, report (T_iters - T_1)/(iters-1)."""
    import time
    import jax
    from jax.sharding import Mesh, NamedSharding, PartitionSpec
    from jax.experimental.shard_map import shard_map
    import concourse.mybir as mybir
    from concourse import bass2jax
    from concourse.bass2jax import _bass_exec_p, install_neuronx_cc_hook

    install_neuronx_cc_hook()
    timings = {}
    for n_rep in (1, iters):
        nc = _build_nc(n_rep, phases)
        in_names, out_names, out_avals, zero_outs = [], [], [], []
        partition_name = (nc.partition_id_tensor.name
                          if nc.partition_id_tensor else None)
        for alloc in nc.m.functions[0].allocations:
            if not isinstance(alloc, mybir.MemoryLocationSet):
                continue
            name = alloc.memorylocations[0].name
            if alloc.kind == "ExternalInput":
                if name != partition_name:
                    in_names.append(name)
            elif alloc.kind == "ExternalOutput":
                shape = tuple(alloc.tensor_shape)
                dtype = mybir.dt.np(alloc.dtype)
                out_names.append(name)
                out_avals.append(jax.core.ShapedArray(shape, dtype))
                zero_outs.append(np.zeros(shape, dtype))
        n_params = len(in_names)
        n_outs = len(out_avals)
        all_in_names = list(in_names) + list(out_names)
        if partition_name is not None:
            all_in_names.append(partition_name)

        def _body(*args, _nc=nc, _avals=tuple(out_avals),
                  _innames=tuple(all_in_names), _outnames=tuple(out_names),
                  _pname=partition_name):
            operands = list(args)
            if _pname is not None:
                operands.append(bass2jax.partition_id_tensor())
            outs = _bass_exec_p.bind(
                *operands, out_avals=_avals, in_names=_innames,
                out_names=_outnames, lowering_input_output_aliases=(),
                sim_require_finite=True, sim_require_nnan=True, nc=_nc)
            return tuple(outs)

        devices = jax.devices()[:NCORES]
        mesh = Mesh(np.asarray(devices), ("core",))
        in_specs = (PartitionSpec("core"),) * (n_params + n_outs)
        out_specs = (PartitionSpec("core"),) * n_outs
        per_core = [[np.asarray(m[name]) for name in in_names]
                    for m in in_maps]
        concat_in = [np.concatenate([per_core[c][i] for c in range(NCORES)],
                                    axis=0) for i in range(n_params)]
        shard = NamedSharding(mesh, PartitionSpec("core"))
        dev_in = [jax.device_put(a, shard) for a in concat_in]

        donate = tuple(range(n_params, n_params + n_outs))
        fn = jax.jit(shard_map(_body, mesh=mesh, in_specs=in_specs,
                               out_specs=out_specs, check_rep=False),
                     donate_argnums=donate, keep_unused=True)
        # pre-stage donated output buffers on device (sharded) so the
        # timed region is dispatch + execute only; value-varying fills
        # defeat any result-memoization in the execution path
        zs_dev = []
        for i in range(reps + 1):
            zs = [np.full((NCORES * z.shape[0], *z.shape[1:]), 1e-6 * i,
                          z.dtype) for z in zero_outs]
            zs_dev.append([jax.device_put(z, shard) for z in zs])
        jax.block_until_ready(zs_dev)
        samples = []
        for i in range(reps + 1):
            t0 = time.perf_counter()
            outs = fn(*dev_in, *zs_dev[i])
            jax.block_until_ready(outs)
            dt = time.perf_counter() - t0
            if i > 0:
                samples.append(dt)
        timings[n_rep] = min(samples)
        del zs_dev
    per_iter = (timings[iters] - timings[1]) / (iters - 1)
    return per_iter * 1e9, timings[1] * 1e9


def kernel(**inputs):
    hp = host_prep(inputs)
    in_maps = []
    for core in range(NCORES):
        ci = core_inputs(hp, core)
        in_maps.append({k: np.ascontiguousarray(v) for k, v in ci.items()})
    trace = bool(int(os.environ.get("KERNEL_TRACE", "0")))
    results = _run(in_maps, trace)
    out = np.zeros((B, T, C), np.float32)
    for core in range(NCORES):
        b, j = core // G, core % G
        out[b, j * TQ:(j + 1) * TQ, :] = results.results[core]["outT"].T
    return out


if __name__ == "__main__":
    import reference
    inputs = reference.setup_inputs()
    out = kernel(**{k: np.asarray(v) for k, v in inputs.items()})
    exp = np.asarray(reference.reference(**inputs))
    err = np.abs(out - exp).max() / np.abs(exp).max()
    print("rel(absmax) err:", err)
